# revision 7
# baseline (speedup 1.0000x reference)
# Multi-head attention (B=4, S=2048, D=1024, H=16, causal) on 8 Trainium2
# NeuronCores via Bass/Tile.
#
# Sharding: 8 cores = 4 batches x 2 head-groups (8 heads each).
# Each core computes, for its (batch, head-group):
#   qT = (Wq_g @ x_q^T) * scale + bq_g   [F=512 feats x S]   (transposed layout)
#   kT =  Wk_g @ x_k^T                    [F x S]
#   v  =  x_v @ Wv_g^T                    [S x F]  (+ ones column per head)
#   per head: scoresT[j,q] = kT_h[:,j] . qT_h[:,q]   (keys on partitions)
#             expT = exp(scoresT) (* mask chunk where needed)
#             outT_h[d,q] (+ denom row) = v_aug_h^T-contraction over keys
#             concatT[f,q] = outT_h / denom
#   partial^T[e,s] = Wo_g^T-contraction over local features
# Host sums the two head-group partials per batch and adds the folded bias.
#
# Bias algebra (exact): k-bias cancels inside softmax (constant per row);
# v-bias and o-bias fold into a host-side output offset since softmax rows
# sum to 1. Only the q-bias is applied on-device.

import numpy as np
from contextlib import ExitStack

import concourse.bass as bass
import concourse.bacc as bacc
import concourse.mybir as mybir
import concourse.tile as tile
from concourse.bass_utils import run_bass_kernel_spmd

AF = mybir.ActivationFunctionType
ALU = mybir.AluOpType
F32 = mybir.dt.float32
F32R = mybir.dt.float32r

B, S, D, H, DK = 4, 2048, 1024, 16, 64
P, SF = 128, 512          # partition tile / free-dim chunk
NH = 8                    # heads per core
F = NH * DK               # 512 local features
NCO = D // P              # 8 contraction chunks
NFP = F // P              # 4 feature-partition chunks
NSF = S // SF             # 4 seq free chunks
NJJ = S // P              # 16 key chunks
NEC = D // P              # 8 output-feature chunks
SCALE = 1.0 / np.sqrt(DK)

MAX_MASK_PATTERNS = 16


def _body(ctx, tc, xq, xk, xv, wq, wk, wv, wo, bq, mm, ones_d, outT, classes, n_pat):
    nc = tc.nc
    persist = ctx.enter_context(tc.tile_pool(name="persist", bufs=1))

    bq_sb = persist.tile([P, NFP], F32, tag="bq")
    nc.sync.dma_start(out=bq_sb, in_=bq.rearrange("(a p) -> p a", p=P))

    mask_sb = []
    for i in range(n_pat):
        mt = persist.tile([P, SF], F32R, tag=f"mask{i}")
        nc.sync.dma_start(out=mt, in_=mm[i])
        mask_sb.append(mt)

    ones_sb = persist.tile([P, DK], F32R, tag="ones")
    nc.sync.dma_start(out=ones_sb, in_=ones_d)

    qT = [persist.tile([P, S], F32R, tag=f"qT{i}", name=f"qT{i}") for i in range(NFP)]
    kT = [persist.tile([P, S], F32R, tag=f"kT{i}", name=f"kT{i}") for i in range(NFP)]
    vA = [persist.tile([P, NH, DK + 1], F32R, tag=f"v{i}", name=f"v{i}") for i in range(NJJ)]
    cT = [persist.tile([P, S], F32R, tag=f"cT{i}", name=f"cT{i}") for i in range(NFP)]

    # ---------- Phase 1a/b: Q and K projections (transposed outputs) ----------
    for pname, xin, w_dram, dst, scale, bias_sb in (
        ("q", xq, wq, qT, SCALE, bq_sb),
        ("k", xk, wk, kT, 1.0, None),
    ):
        with tc.tile_pool(name=f"w_{pname}", bufs=1) as wp, \
             tc.tile_pool(name=f"x_{pname}", bufs=3) as xp, \
             tc.tile_pool(name=f"ps_{pname}", bufs=4, space="PSUM") as pp:
            w_sb = wp.tile([P, NCO, F], F32R, tag="w")
            nc.sync.dma_start(out=w_sb, in_=w_dram.rearrange("(co p) f -> p co f", p=P))
            xr = xin.rearrange("(co p) s -> p co s", p=P)
            for sf in range(NSF):
                xt = xp.tile([P, NCO, SF], F32R, tag="x")
                nc.sync.dma_start(out=xt, in_=xr[:, :, sf * SF:(sf + 1) * SF])
                for fp in range(NFP):
                    ps = pp.tile([P, SF], F32, tag="ps")
                    for co in range(NCO):
                        nc.tensor.matmul(
                            ps, (w_sb[:, co, fp * P:(fp + 1) * P]), (xt[:, co, :]),
                            start=(co == 0), stop=(co == NCO - 1))
                    dst_ap = dst[fp][:, sf * SF:(sf + 1) * SF]
                    if bias_sb is not None:
                        nc.scalar.activation(out=dst_ap, in_=ps, func=AF.Identity,
                                             bias=bias_sb[:, fp:fp + 1], scale=scale)
                    else:
                        nc.scalar.copy(out=dst_ap, in_=ps)

    # ---------- Phase 1c: V projection (normal layout) + ones column ----------
    with tc.tile_pool(name="w_v", bufs=1) as wp, \
         tc.tile_pool(name="x_v", bufs=3) as xp, \
         tc.tile_pool(name="ps_v", bufs=4, space="PSUM") as pp:
        wv_sb = wp.tile([P, NCO, F], F32R, tag="w")
        nc.sync.dma_start(out=wv_sb, in_=wv.rearrange("(co p) f -> p co f", p=P))
        xr = xv.rearrange("(co p) s -> p co s", p=P)
        for sp in range(NJJ):
            xt = xp.tile([P, NCO, P], F32R, tag="x")
            nc.sync.dma_start(out=xt, in_=xr[:, :, sp * P:(sp + 1) * P])
            ps = pp.tile([P, F], F32, tag="ps")
            for co in range(NCO):
                nc.tensor.matmul(ps, (xt[:, co, :]), (wv_sb[:, co, :]),
                                 start=(co == 0), stop=(co == NCO - 1))
            nc.vector.tensor_copy(out=vA[sp][:, :, 0:DK],
                                  in_=ps.rearrange("p (h d) -> p h d", h=NH))
            nc.sync.dma_start(out=vA[sp][:, :, DK:DK + 1], in_=ones_d[:, 0:NH])

    # ---------- Phase 2: attention ----------
    with tc.tile_pool(name="sps", bufs=3, space="PSUM") as sp_pool, \
         tc.tile_pool(name="ops", bufs=2, space="PSUM") as op_pool, \
         tc.tile_pool(name="rps", bufs=2, space="PSUM") as rp_pool, \
         tc.tile_pool(name="et", bufs=4) as ep, \
         tc.tile_pool(name="dn", bufs=3) as dnp, \
         tc.tile_pool(name="rc", bufs=3) as rcp, \
         tc.tile_pool(name="stg", bufs=2) as stgp:
        for hl in range(NH):
            fc, odd = divmod(hl, 2)
            r0 = odd * DK
            for qi in range(NSF):
                used = [jj for jj in range(NJJ) if classes[(qi, jj)] != "skip"]
                qv = qT[fc][r0:r0 + DK, qi * SF:(qi + 1) * SF]
                po = op_pool.tile([P, SF], F32, tag="po")
                for i, jj in enumerate(used):
                    ps = sp_pool.tile([P, SF], F32, tag="ps")
                    nc.tensor.matmul(ps, (kT[fc][r0:r0 + DK, jj * P:(jj + 1) * P]),
                                     (qv), start=True, stop=True)
                    et = ep.tile([P, SF], F32R, tag="et")
                    nc.scalar.activation(out=et, in_=ps, func=AF.Exp)
                    cl = classes[(qi, jj)]
                    if cl != "free":
                        nc.vector.tensor_mul(et, et, mask_sb[cl])
                    nc.tensor.matmul(po[0:DK + 1, :], (vA[jj][:, hl, :]), (et),
                                     start=(i == 0), stop=(i == len(used) - 1))
                # denominator: row DK of po. Keep it on partition 64 to stay
                # partition-aligned for ACT, then broadcast via a K=1 matmul.
                dn = dnp.tile([P, SF], F32R, tag="dn")
                nc.scalar.copy(out=dn[DK:DK + 1, :], in_=po[DK:DK + 1, :])
                rp = rp_pool.tile([DK, SF], F32, tag="rp")
                nc.tensor.matmul(rp, (ones_sb[DK:DK + 1, 0:DK]), (dn[DK:DK + 1, :]),
                                 start=True, stop=True)
                rc = rcp.tile([DK, SF], F32, tag="rc")
                nc.vector.reciprocal(rc, rp)
                if odd == 0:
                    nc.vector.tensor_tensor(
                        out=cT[fc][0:DK, qi * SF:(qi + 1) * SF],
                        in0=po[0:DK, :], in1=rc, op=ALU.mult)
                else:
                    stg = stgp.tile([DK, SF], F32R, tag="stg")
                    nc.vector.tensor_tensor(out=stg, in0=po[0:DK, :], in1=rc,
                                            op=ALU.mult)
                    nc.sync.dma_start(out=cT[fc][DK:2 * DK, qi * SF:(qi + 1) * SF],
                                      in_=stg)

    # ---------- Phase 3: output projection ----------
    with tc.tile_pool(name="w_o", bufs=1) as wp, \
         tc.tile_pool(name="ps_o", bufs=4, space="PSUM") as pp, \
         tc.tile_pool(name="ot", bufs=4) as otp:
        wo_sb = wp.tile([P, NFP, D], F32R, tag="w")
        nc.sync.dma_start(out=wo_sb, in_=wo.rearrange("(fc p) e -> p fc e", p=P))
        for ec in range(NEC):
            for sc in range(NSF):
                ps = pp.tile([P, SF], F32, tag="ps")
                for fc in range(NFP):
                    nc.tensor.matmul(ps, (wo_sb[:, fc, ec * P:(ec + 1) * P]),
                                     (cT[fc][:, sc * SF:(sc + 1) * SF]),
                                     start=(fc == 0), stop=(fc == NFP - 1))
                ot = otp.tile([P, SF], F32, tag="ot")
                nc.scalar.copy(out=ot, in_=ps)
                nc.sync.dma_start(
                    out=outT[ec * P:(ec + 1) * P, sc * SF:(sc + 1) * SF], in_=ot)


def build(classes, n_pat):
    nc = bacc.Bacc("TRN2", target_bir_lowering=False, debug=False)

    def din(name, shape, dt=F32R):
        return nc.dram_tensor(name, shape, dt, kind="ExternalInput").ap()

    xq, xk, xv = din("xqT", (D, S)), din("xkT", (D, S)), din("xvT", (D, S))
    wq, wk, wv = din("wqT", (D, F)), din("wkT", (D, F)), din("wvT", (D, F))
    wo = din("woT", (F, D))
    bq = din("bq", (F,), F32)
    mm = din("mmix", (max(n_pat, 1), P, SF))
    ones_d = din("ones_d", (P, DK))
    outT = nc.dram_tensor("outT", (D, S), F32, kind="ExternalOutput").ap()

    with tile.TileContext(nc) as tc:
        with ExitStack() as ctx:
            _body(ctx, tc, xq, xk, xv, wq, wk, wv, wo, bq, mm, ones_d, outT,
                  classes, n_pat)
    nc.compile()
    return nc


def classify_mask(mask2d):
    """Per (qi, jj) chunk of the [S, S] bool mask: 'skip' (all False),
    'free' (all True), or a dedup'd mixed-pattern id. Patterns are stored
    transposed ([keys, queries]) as f32 multiplicative masks."""
    classes = {}
    patterns = []
    pattern_keys = {}
    for qi in range(NSF):
        for jj in range(NJJ):
            chunk = mask2d[qi * SF:(qi + 1) * SF, jj * P:(jj + 1) * P]
            if not chunk.any():
                classes[(qi, jj)] = "skip"
            elif chunk.all():
                classes[(qi, jj)] = "free"
            else:
                key = chunk.tobytes()
                if key not in pattern_keys:
                    pattern_keys[key] = len(patterns)
                    patterns.append(np.ascontiguousarray(chunk.T).astype(np.float32))
                classes[(qi, jj)] = pattern_keys[key]
    return classes, patterns


def _head_index(g):
    # local feature f = hl*64 + d  maps to reference row  d*16 + (8g + hl)
    hl = np.arange(NH)
    d = np.arange(DK)
    return (d[None, :] * H + (NH * g + hl)[:, None]).reshape(-1)


def make_in_maps(query, key, value, w_q, b_q, w_k, w_v, w_o, patterns):
    n_pat = max(len(patterns), 1)
    mm = np.zeros((n_pat, P, SF), np.float32)
    for i, pat in enumerate(patterns):
        mm[i] = pat
    in_maps = []
    for c in range(8):
        b, g = divmod(c, 2)
        idx = _head_index(g)
        in_maps.append({
            "xqT": np.ascontiguousarray(query[b].T, np.float32),
            "xkT": np.ascontiguousarray(key[b].T, np.float32),
            "xvT": np.ascontiguousarray(value[b].T, np.float32),
            "wqT": np.ascontiguousarray(w_q[idx, :].T, np.float32),
            "wkT": np.ascontiguousarray(w_k[idx, :].T, np.float32),
            "wvT": np.ascontiguousarray(w_v[idx, :].T, np.float32),
            "woT": np.ascontiguousarray(w_o[:, F * g:F * (g + 1)].T, np.float32),
            "bq": (b_q[idx] * SCALE).astype(np.float32),
            "mmix": mm,
            "ones_d": np.ones((P, DK), np.float32),
        })
    return in_maps


def fold_output_bias(b_o, b_v, w_o):
    # softmax rows sum to 1 => v-bias contributes  w_o @ bv_concat  exactly.
    bv_concat = np.zeros(D, np.float32)
    for g in range(2):
        bv_concat[F * g:F * (g + 1)] = b_v[_head_index(g)]
    return (b_o + w_o @ bv_concat).astype(np.float32)


def _reference_numpy(query, key, value, mask, w_q, b_q, w_k, b_k, w_v, b_v,
                     w_o, b_o):
    # exact fallback mirroring reference.py (chunked, fp32/f64-free)
    Bn, Sn, Dn = query.shape
    Hn = H
    DKn = Dn // Hn
    q = query @ w_q.T + b_q
    k = key @ w_k.T + b_k
    v = value @ w_v.T + b_v

    def split(x):
        return x.reshape(Bn, Sn, DKn, Hn).transpose(0, 3, 1, 2)

    q, k, v = split(q), split(k), split(v)
    m = np.broadcast_to(np.asarray(mask), (1, 1, Sn, Sn))[0, 0]
    out = np.empty((Bn, Sn, Hn * DKn), np.float32)
    scale = 1.0 / np.sqrt(DKn)
    for b in range(Bn):
        for h in range(Hn):
            s = (q[b, h] @ k[b, h].T) * scale
            s = np.where(m, s, -np.inf)
            s -= s.max(axis=-1, keepdims=True)
            e = np.exp(s)
            p = e / e.sum(axis=-1, keepdims=True)
            out[b, :, h * DKn:(h + 1) * DKn] = p @ v[b, h]
    return out @ w_o.T + b_o


_CACHE = {}

# test harness hooks: set TRACE=True before calling kernel() to profile;
# the raw BassKernelResults of the last run lands in LAST_RESULTS.
TRACE = False
LAST_RESULTS = None


def kernel(query, key, value, mask, w_q, b_q, w_k, b_k, w_v, b_v, w_o, b_o):
    query = np.asarray(query, np.float32)
    key = np.asarray(key, np.float32)
    value = np.asarray(value, np.float32)
    shapes_ok = (query.shape == (B, S, D) and key.shape == (B, S, D)
                 and value.shape == (B, S, D)
                 and np.asarray(mask).shape[-2:] == (S, S)
                 and w_q.shape == (D, D) and w_o.shape == (D, D))
    if not shapes_ok:
        return _reference_numpy(query, key, value, mask, w_q, b_q, w_k, b_k,
                                w_v, b_v, w_o, b_o)

    mask2d = np.broadcast_to(np.asarray(mask), (1, 1, S, S))[0, 0].astype(bool)
    classes, patterns = classify_mask(mask2d)
    if len(patterns) > MAX_MASK_PATTERNS or any(
            all(classes[(qi, jj)] == "skip" for jj in range(NJJ))
            for qi in range(NSF)):
        return _reference_numpy(query, key, value, mask, w_q, b_q, w_k, b_k,
                                w_v, b_v, w_o, b_o)

    ckey = tuple(sorted(classes.items())) + (len(patterns),)
    if ckey not in _CACHE:
        _CACHE[ckey] = build(classes, len(patterns))
    nc = _CACHE[ckey]

    in_maps = make_in_maps(query, key, value,
                           np.asarray(w_q, np.float32), np.asarray(b_q, np.float32),
                           np.asarray(w_k, np.float32), np.asarray(w_v, np.float32),
                           np.asarray(w_o, np.float32), patterns)
    res = run_bass_kernel_spmd(nc, in_maps, core_ids=list(range(8)),
                               trace=TRACE)
    global LAST_RESULTS
    LAST_RESULTS = res

    bo_eff = fold_output_bias(np.asarray(b_o, np.float32),
                              np.asarray(b_v, np.float32),
                              np.asarray(w_o, np.float32))
    out = np.empty((B, S, D), np.float32)
    for b in range(B):
        acc = res.results[2 * b]["outT"].T + res.results[2 * b + 1]["outT"].T
        out[b] = acc + bo_eff
    return out
